# revision 59
# baseline (speedup 1.0000x reference)
"""Single-head causal attention (B=4, T=4096, E=1024, H=128) on 8 TRN2 cores.

Sharding: 2 cores per batch, "folded triangle" split of the causal work.
Chunk0 = queries [0,2048), chunk1 = [2048,4096).
  core (b, 0): TRI : chunk0 q vs k in [0, q]        (causal triangle)
               RECT: chunk1 q vs k in [0, 1024)     (no mask)
  core (b, 1): TRI : chunk1 q vs k in [2048, q]     (causal triangle)
               RECT: chunk1 q vs k in [1024, 2048)  (no mask)
All 8 cores run the identical SPMD program; only the data differs. Outputs
are unnormalized accumulators acc = P@V and row-sums l = P@1; the host sums
chunk1 partials and normalizes y = acc/(32*l).

fp8 fast path: x and W ship as fp8e4 with W pre-scaled by 32 (so W' is
U(-1,1), in e4m3's clean range), and projections run as fp8 DoubleRow
matmuls (2 E-chunks per pass, 2x PE rate). V^T is produced directly by the
PE (x-slice as the stationary operand) so no transposes are needed. Q/K
stay bf16 for the S = K^T Q matmul (contraction is 128, so DoubleRow cannot
pair there). The exp runs on ScalarE reading S straight from PSUM at scale
1/(1024*sqrt(128)) and writes P^T as fp8e4; A@V and the row-sum matmuls are
then fp8 DoubleRow over paired k-tiles. The causal mask is applied POST-exp
on the (otherwise idle) GPSIMD engine as an affine_select fill of 0 on pt;
fully-masked diagonal columns are skipped in both the S matmuls and the
exp. Selective precision: the first 512 rows of each core's triangle chunk
(the rows whose softmax has too few terms to average out fp8 noise) use a
bf16 path end-to-end -- bf16 x/W block-0 projections (qtBF/ktBF/vsbBF) and
bf16 P/AV/l for that group -- which recovers reference-level accuracy
(rel err ~3.7e-3 vs 5.2e-2 all-fp8).

Schedule: attention is a software-pipelined stream of 36 pair-jobs
(S -> exp -> mask -> AV+l), group order (qb4, qb1, qb2, qb3, qb5, qb6,
then qb0's two bf16 jobs interleaved inside qb7's four fp8 jobs so the
fp8 stream hides the wait for the bf16 trio's evacuations), with
projection units interleaved one-per-slot. PSUM: psS 2x2
banks (S tiles + bf16-unit scratch), psY 2x1 (AV accumulators + V^T units),
psT 2x1 (q/k proj + row-sum accumulators); unit placement follows a strict
rotation audit so no long-lived tile ever blocks the pool rotation. x
blocks are prefetched up-front on one DMA queue in first-use order.
"""

import sys

if "/opt/trn_rl_repo" not in sys.path:
    sys.path.insert(0, "/opt/trn_rl_repo")

import numpy as np
import ml_dtypes

import concourse.bacc as bacc
import concourse.bass as bass
import concourse.mybir as mybir
from concourse import masks, tile
from concourse.bass_utils import run_bass_kernel_spmd

E = 1024
H = 128
T = 4096
CH = 2048            # chunk length
TQ = 4096            # q tokens per core: [tri own-chunk 2048 | rect chunk1 2048]
RK = 1024            # rect-k region length
NKT_RK = RK // 128   # 8 k-tiles in the rect-k region
WS = 32.0            # host pre-scale on W (and so on q', k', v')
SCALE = 1.0 / np.sqrt(np.float32(H))
SCALE8 = SCALE / (WS * WS)   # exp scale for q'.k' = 1024 * q.k
NEG = -1.0e9                 # masked: NEG*SCALE8 ~ -8.6e3 -> exp == 0

F32 = mybir.dt.float32
BF16 = mybir.dt.bfloat16
F8 = mybir.dt.float8e4
DR = mybir.MatmulPerfMode.DoubleRow

EC = E // 128       # 8 contraction chunks for the projections
NB_Q = TQ // 512    # 8 q blocks
NB_RK = RK // 512   # 2 rect-k blocks

_CACHED = {}
TRACE = False
TRACE_CORES = None
LAST_RESULTS = None


def _build(loop_n=None):
    nc = bacc.Bacc("TRN2", target_bir_lowering=False, debug=False, num_devices=8)
    # host-tiled: [128p, tb, ec, 512] (fp8e4)
    xq_in = nc.dram_tensor("xq_in", [128, NB_Q, EC, 512], F8, kind="ExternalInput").ap()
    xrk_in = nc.dram_tensor("xrk_in", [128, NB_RK, EC, 512], F8, kind="ExternalInput").ap()
    w3_in = nc.dram_tensor("w3_in", [128, 3, EC, H], F8, kind="ExternalInput").ap()
    xb0_bf_in = nc.dram_tensor("xb0_bf_in", [128, EC, 512], BF16, kind="ExternalInput").ap()
    w3_bf_in = nc.dram_tensor("w3_bf_in", [128, 3, EC, H], BF16, kind="ExternalInput").ap()
    acc_out = nc.dram_tensor("acc_out", [H, TQ], F32, kind="ExternalOutput").ap()
    warm_out = nc.dram_tensor("warm_out", [1, 1], F32, kind="ExternalOutput").ap()
    l_out = nc.dram_tensor("l_out", [1, TQ], F32, kind="ExternalOutput").ap()

    import contextlib

    with tile.TileContext(nc) as tc:
        loop_cm = tc.For_i(0, loop_n, 1) if loop_n else contextlib.nullcontext()
        with (
            tc.tile_pool(name="const", bufs=1) as constp,
            tc.tile_pool(name="wpool", bufs=1) as wpool,
            tc.tile_pool(name="xpool", bufs=1) as xpool,
            tc.tile_pool(name="proj", bufs=1) as projp,
            tc.tile_pool(name="ppool", bufs=8) as ppool,
            tc.tile_pool(name="outp", bufs=4) as outp,
            tc.tile_pool(name="psS", bufs=2, space="PSUM") as psS,
            tc.tile_pool(name="psY", bufs=2, space="PSUM") as psY,
            tc.tile_pool(name="psT", bufs=2, space="PSUM") as psT,
            loop_cm,
        ):
            # ---- input DMAs on two queues (SP + DVE) for parallel streams
            xtiles = {}

            def xdma(eng, key, src_, tb):
                xb = xpool.tile([128, EC, 512], F8, tag=f"x{key}", name=f"x{key}")
                eng.dma_start(xb[:], src_[:, tb])
                xtiles[key] = xb

            w3 = wpool.tile([128, 3, EC, H], F8, tag="w3")
            nc.sync.dma_start(w3[:], w3_in)
            wtiles = {name: w3[:, i] for i, name in enumerate(("q", "k", "v"))}
            xdma(nc.sync, "q4", xq_in, 4)
            xdma(nc.sync, "rk0", xrk_in, 0)
            xdma(nc.sync, "rk1", xrk_in, 1)
            xdma(nc.sync, "q0", xq_in, 0)
            xq1 = xpool.tile([128, EC, 512], F8, tag="xq1", name="xq1")
            nc.sync.dma_start(xq1[:, :4, :], xq_in[:, 1, :4, :])
            nc.sync.dma_start(xq1[:, 4:, :], xq_in[:, 1, 4:, :])
            xtiles["q1"] = xq1
            xdma(nc.sync, "q2", xq_in, 2)
            w3bf = wpool.tile([128, 3, EC, H], BF16, tag="w3bf")
            nc.sync.dma_start(w3bf[:], w3_bf_in)
            xb0bf = xpool.tile([128, EC, 512], BF16, tag="xb0bf", name="xb0bf")
            nc.sync.dma_start(xb0bf[:, :4, :], xb0_bf_in[:, :4, :])
            nc.sync.dma_start(xb0bf[:, 4:, :], xb0_bf_in[:, 4:, :])
            wbf = {name: w3bf[:, i] for i, name in enumerate(("q", "k", "v"))}
            xdma(nc.sync, "q3", xq_in, 3)
            xdma(nc.sync, "q5", xq_in, 5)
            xdma(nc.sync, "q6", xq_in, 6)
            xdma(nc.sync, "q7", xq_in, 7)

            # ---- constants (Pool engine; overlaps the DMAs) ----
            ones_f = constp.tile([128, 1], F32, tag="ones32")
            nc.gpsimd.memset(ones_f[:], 1.0)
            ones8 = constp.tile([128, 2, 32], F8, tag="ones8")
            nc.gpsimd.memset(ones8[:], 1.0)
            ones_bf = constp.tile([128, 1], BF16, tag="ones_bf")
            nc.vector.tensor_copy(ones_bf[:], ones_f[:])
            warm = constp.tile([1, 1], F32, tag="warm")
            nc.scalar.activation(
                warm[:], ones_f[:1, :1], mybir.ActivationFunctionType.Exp, scale=1.0
            )
            nc.sync.dma_start(warm_out, warm[:])

            # ---- projection targets ----
            # QT [128h, TQ]; KT [128h, 3072] (bf16); V^T [token, k-tile, h] f8
            # kv tile space: tiles 0..7 = rect-k, 8..23 = tri chunk
            qt = projp.tile([128, TQ], BF16, tag="qt")
            kt = projp.tile([128, RK + CH], BF16, tag="kt")
            ktBF = projp.tile([128, 512], BF16, tag="ktBF")   # bf16 k, tiles 8-11
            vsb = projp.tile([128, NKT_RK + CH // 128, 128], F8, tag="v")
            vsbBF = projp.tile([128, 4, 128], BF16, tag="vbf")

            def qk_unit(name, xb, dcol):
                """f8 DoubleRow projection of q or k; evacuate to qt/kt."""
                ps = psT.tile([128, 512], F32, tag="psproj")
                for e2 in range(EC // 2):
                    nc.tensor.matmul(
                        ps[:], wtiles[name][:, 2 * e2 : 2 * e2 + 2, :],
                        xb[:, 2 * e2 : 2 * e2 + 2, :],
                        start=(e2 == 0), stop=(e2 == EC // 2 - 1),
                        perf_mode=DR,
                    )
                dst = kt if name == "k" else qt
                nc.vector.tensor_copy(dst[:, dcol : dcol + 512], ps[:])

            def v_unit(xb, kt0):
                """f8 V^T computed directly: out[token, h] = x_sliceT @ Wv.
                Allocates from psY: v-units interleave cleanly with ys tiles."""
                ps = psY.tile([128, 512], F32, tag="y", name="psv")
                for tj in range(4):
                    for e2 in range(EC // 2):
                        nc.tensor.matmul(
                            ps[:, 128 * tj : 128 * (tj + 1)],
                            xb[:, 2 * e2 : 2 * e2 + 2, 128 * tj : 128 * (tj + 1)],
                            wtiles["v"][:, 2 * e2 : 2 * e2 + 2, :],
                            start=(e2 == 0), stop=(e2 == EC // 2 - 1),
                            perf_mode=DR,
                        )
                nc.vector.tensor_copy(vsb[:, kt0 : kt0 + 4, :], ps[:])

            # bf16 block-0 units, each emitted as 4 chunks spread over
            # consecutive job slots (the psum alloc happens in chunk 0 at the
            # rotation-audited slot; later chunks add matmuls only).
            _bfst = {}

            def qkbf_chunk(name, dst, c):
                if c == 0:
                    _bfst[name] = psT.tile([128, 512], F32, tag="psproj",
                                           name=f"ps{name}bf")
                ps = _bfst[name]
                for ec in (2 * c, 2 * c + 1):
                    nc.tensor.matmul(ps[:], wbf[name][:, ec, :],
                                     xb0bf[:, ec, :],
                                     start=(ec == 0), stop=(ec == EC - 1))
                if c == 3:
                    nc.vector.tensor_copy(dst, ps[:])

            def qbf_chunk(c):
                qkbf_chunk("q", qt[:, 0:512], c)

            def kbf_chunk(c):
                qkbf_chunk("k", ktBF[:], c)

            def vbf_chunk(c):
                if c == 0:
                    _bfst["v"] = psY.tile([128, 512], F32, tag="y",
                                          name="psvbf")
                ps = _bfst["v"]
                tj = c
                for ec in range(EC):
                    nc.tensor.matmul(
                        ps[:, 128 * tj : 128 * (tj + 1)],
                        xb0bf[:, ec, 128 * tj : 128 * (tj + 1)],
                        wbf["v"][:, ec, :],
                        start=(ec == 0), stop=(ec == EC - 1))
                if c == 3:
                    nc.vector.tensor_copy(vsbBF[:], ps[:])

            # ---- attention jobs; group order keeps the bf16 block off the
            # DMA critical path and respects the psT rotation audit ----
            # job = (qb, k0, d0, k1, d1, first_in_block, last_in_block)
            gjobs = {}
            for qb in range(8):
                gjobs[qb] = []
                if qb < 4:  # triangle over kv tiles 8..(8+4qb+4)
                    kts = []
                    for j in range(4 * qb + 4):
                        dp = j - 4 * qb if j >= 4 * qb else None
                        kts.append((NKT_RK + j, dp))
                    for i in range(len(kts) // 2):
                        (k0, d0), (k1, d1) = kts[2 * i], kts[2 * i + 1]
                        gjobs[qb].append((qb, k0, d0, k1, d1, i == 0,
                                          2 * i + 2 == len(kts)))
                else:  # rect over kv tiles 0..7
                    for i in range(NKT_RK // 2):
                        gjobs[qb].append((qb, 2 * i, None, 2 * i + 1, None,
                                          i == 0, 2 * i + 2 == NKT_RK))
            # tail: interleave qb0's two bf16 jobs inside qb7 so the fp8
            # stream hides the wait for the bf16 trio's evacuations
            jobs = (gjobs[4] + gjobs[1] + gjobs[2] + gjobs[3] + gjobs[5]
                    + gjobs[6] + [gjobs[7][0], gjobs[0][0], gjobs[7][1],
                                  gjobs[0][1], gjobs[7][2], gjobs[7][3]])

            n = len(jobs)
            ss_t = [None] * n
            pt_t = [None] * n
            ybank = {}

            def s_stage(j):
                qb, k0, d0, k1, d1, _, _ = jobs[j]
                ss = psS.tile([128, 2, 512], F32, tag="s")
                ss_t[j] = ss
                qs = qt[:, 512 * qb : 512 * (qb + 1)]
                if qb == 0:
                    kt0 = ktBF[:, 128 * (k0 - NKT_RK) : 128 * (k0 - NKT_RK + 1)]
                    kt1 = ktBF[:, 128 * (k1 - NKT_RK) : 128 * (k1 - NKT_RK + 1)]
                else:
                    kt0 = kt[:, 128 * k0 : 128 * (k0 + 1)]
                    kt1 = kt[:, 128 * k1 : 128 * (k1 + 1)]
                # fully-masked diag columns [0, 128*d) are never computed:
                # the Pool mask fill overwrites them in pt afterwards.
                c0 = 128 * d0 if d0 else 0
                c1 = 128 * d1 if d1 else 0
                nc.tensor.matmul(ss[:, 0, c0:], kt0, qs[:, c0:],
                                 start=True, stop=True)
                nc.tensor.matmul(ss[:, 1, c1:], kt1, qs[:, c1:],
                                 start=True, stop=True)
                pt = ppool.tile([128, 2, 512], BF16 if qb == 0 else F8, tag="pt")
                pt_t[j] = pt
                if d1 is not None and d1 >= 2:
                    # (2,3) diag pair: one narrowed exp over both halves'
                    # live region [256, 512) + [384->256...]: use col range
                    # [c0, 512) of both halves in a single activation
                    nc.scalar.activation(
                        pt[:, :, c0:], ss[:, :, c0:],
                        mybir.ActivationFunctionType.Exp, scale=SCALE8)
                else:
                    nc.scalar.activation(
                        pt[:], ss[:], mybir.ActivationFunctionType.Exp,
                        scale=SCALE8)
                # causal mask applied post-exp on the (idle) Pool engine:
                # zero pt where q < k + 128*d; only cols [0, 128*(d+1)) affected
                if d0 is not None:
                    w = 128 * (d0 + 1)
                    nc.gpsimd.affine_select(
                        out=pt[:, 0, :w], in_=pt[:, 0, :w],
                        compare_op=mybir.AluOpType.is_ge,
                        fill=0.0, base=-128 * d0,
                        pattern=[[1, w]], channel_multiplier=-1,
                    )
                if d1 is not None:
                    w = 128 * (d1 + 1)
                    nc.gpsimd.affine_select(
                        out=pt[:, 1, :w], in_=pt[:, 1, :w],
                        compare_op=mybir.AluOpType.is_ge,
                        fill=0.0, base=-128 * d1,
                        pattern=[[1, w]], channel_multiplier=-1,
                    )

            def av_stage(j):
                qb, k0, d0, k1, d1, first, last = jobs[j]
                if first:
                    ys = psY.tile([128, 512], F32, tag="y", name=f"ys{qb}")
                    ls = psT.tile([32, 512], F32, tag="psproj", name=f"ls{qb}")
                    ybank[qb] = (ys, ls)
                ys, ls = ybank[qb]
                pt = pt_t[j]
                if qb == 0:
                    nc.tensor.matmul(ys[:], vsbBF[:, k0 - NKT_RK, :], pt[:, 0, :],
                                     start=first, stop=False)
                    nc.tensor.matmul(ys[:], vsbBF[:, k1 - NKT_RK, :], pt[:, 1, :],
                                     start=False, stop=last)
                    nc.tensor.matmul(ls[:1, :], ones_bf[:], pt[:, 0, :],
                                     start=first, stop=False)
                    nc.tensor.matmul(ls[:1, :], ones_bf[:], pt[:, 1, :],
                                     start=False, stop=last)
                else:
                    nc.tensor.matmul(ys[:], vsb[:, k0 : k0 + 2, :], pt[:],
                                     start=first, stop=last, perf_mode=DR)
                    nc.tensor.matmul(ls[:], ones8[:], pt[:],
                                     start=first, stop=last, perf_mode=DR)
                if last:
                    yo = outp.tile([128, 512], F32, tag="yo")
                    nc.vector.tensor_copy(yo[:], ys[:])
                    nc.sync.dma_start(acc_out[:, 512 * qb : 512 * (qb + 1)], yo[:])
                    lo = outp.tile([1, 512], F32, tag="lo")
                    nc.vector.tensor_copy(lo[:], ls[:1, :])
                    nc.sync.dma_start(l_out[:, 512 * qb : 512 * (qb + 1)], lo[:])

            # unit schedule (see psT rotation audit): kv chunks at group-last
            # slots, q units at +1 after each ls allocation.
            before_s = {
                4: [lambda: qk_unit("q", xtiles["q1"], 512)],
                8: [lambda: qk_unit("q", xtiles["q2"], 1024)],
                14: [lambda: qk_unit("q", xtiles["q3"], 1536)],
                22: [lambda: qk_unit("q", xtiles["q5"], 2560)],
                26: [lambda: qk_unit("q", xtiles["q6"], 3072)],
                30: [lambda: qk_unit("q", xtiles["q7"], 3584)],
            }
            after_av = {
                2: [lambda: v_unit(xtiles["q0"], NKT_RK)],
                3: [lambda: qk_unit("k", xtiles["q1"], RK + 512),
                    lambda: v_unit(xtiles["q1"], NKT_RK + 4)],
                7: [lambda: qk_unit("k", xtiles["q2"], RK + 1024),
                    lambda: v_unit(xtiles["q2"], NKT_RK + 8)],
                13: [lambda: qk_unit("k", xtiles["q3"], RK + 1536),
                     lambda: v_unit(xtiles["q3"], NKT_RK + 12)],
                21: [lambda: qbf_chunk(0)],
                22: [lambda: qbf_chunk(1)],
                23: [lambda: qbf_chunk(2)],
                24: [lambda: qbf_chunk(3)],
                25: [lambda: kbf_chunk(0)],
                26: [lambda: kbf_chunk(1), lambda: vbf_chunk(0)],
                27: [lambda: kbf_chunk(2), lambda: vbf_chunk(1)],
                28: [lambda: kbf_chunk(3), lambda: vbf_chunk(2)],
                29: [lambda: vbf_chunk(3)],
            }

            # prologue: q(b4) + rect-k region (all f8), first S pair, then
            # the b0 f8 k unit (its V rides at after_av[3] in the psY slot).
            qk_unit("q", xtiles["q4"], 2048)
            # k(rk0) with split evacuation: s(0) only needs kt[0:256]
            ps_k0 = psT.tile([128, 512], F32, tag="psproj", name="psk0")
            for e2 in range(EC // 2):
                nc.tensor.matmul(
                    ps_k0[:], wtiles["k"][:, 2 * e2 : 2 * e2 + 2, :],
                    xtiles["rk0"][:, 2 * e2 : 2 * e2 + 2, :],
                    start=(e2 == 0), stop=(e2 == EC // 2 - 1), perf_mode=DR)
            nc.vector.tensor_copy(kt[:, 0:256], ps_k0[:, :256])
            nc.vector.tensor_copy(kt[:, 256:512], ps_k0[:, 256:])
            s_stage(0)
            s_stage(1)
            v_unit(xtiles["rk0"], 0)
            qk_unit("k", xtiles["rk1"], 512)
            v_unit(xtiles["rk1"], 4)
            qk_unit("k", xtiles["q0"], RK)
            av_stage(0)
            for j in range(1, n):
                if j + 1 < n:
                    for u in before_s.get(j + 1, []):
                        u()
                    s_stage(j + 1)
                av_stage(j)
                for u in after_av.get(j, []):
                    u()

    nc.compile()
    return nc


def _prep_x(xpart):
    """[Tpart, E] f32 -> fp8e4 tiled [128, tb, ec, 512] host layout."""
    tb = xpart.shape[0] // 512
    a = xpart.T.astype(ml_dtypes.float8_e4m3)       # [E, Tpart]
    a = a.reshape(EC, 128, tb, 512).transpose(1, 2, 0, 3)
    return np.ascontiguousarray(a)


def _prep_w(w, dt=None):
    """[H, E] f32 -> [128, ec, H] (32 * w.T chunked) in dt (default fp8e4)."""
    a = (w.T * WS).astype(dt or ml_dtypes.float8_e4m3)  # [E, H]
    a = a.reshape(EC, 128, H).transpose(1, 0, 2)
    return np.ascontiguousarray(a)


def _prep_xbf(xpart):
    """[512, E] f32 -> bf16 tiled [128, ec, 512]."""
    a = xpart.T.astype(ml_dtypes.bfloat16)          # [E, 512]
    a = a.reshape(EC, 128, 512).transpose(1, 0, 2)
    return np.ascontiguousarray(a)


def kernel(x_in, Wq, Wk, Wv):
    B, T_, E_ = x_in.shape
    assert (B, T_, E_) == (4, T, E)
    nc = _CACHED.get("nc")
    if nc is None:
        nc = _CACHED["nc"] = _build()

    bf = ml_dtypes.bfloat16
    w3 = np.ascontiguousarray(np.stack([_prep_w(W) for W in (Wq, Wk, Wv)], axis=1))
    w3b = np.ascontiguousarray(
        np.stack([_prep_w(W, bf) for W in (Wq, Wk, Wv)], axis=1))
    in_maps = []
    for c in range(8):
        b, h = c // 2, c % 2
        xb = np.asarray(x_in[b], dtype=np.float32)
        c0, c1 = xb[:CH], xb[CH:]
        own = c0 if h == 0 else c1
        xq = np.concatenate([own, c1], axis=0)        # [4096, E]
        rk = xb[0:RK] if h == 0 else xb[RK : 2 * RK]  # [1024, E]
        in_maps.append(
            {"xq_in": _prep_x(xq), "xrk_in": _prep_x(rk),
             "xb0_bf_in": _prep_xbf(xq[:512]),
             "w3_in": w3, "w3_bf_in": w3b}
        )

    kw = {}
    if TRACE:
        kw = {"trace": True, "trace_cores": TRACE_CORES}
    res = run_bass_kernel_spmd(nc, in_maps, core_ids=list(range(8)), **kw)
    global LAST_RESULTS
    LAST_RESULTS = res

    y = np.empty((B, T, H), dtype=np.float32)
    inv_ws = 1.0 / WS
    for b in range(4):
        r0, r1 = res.results[2 * b], res.results[2 * b + 1]
        a0, l0 = r0["acc_out"], r0["l_out"][0]
        a1, l1 = r1["acc_out"], r1["l_out"][0]
        y[b, :CH] = (a0[:, :CH] * inv_ws / l0[:CH]).T
        acc = a0[:, CH:] + a1[:, :CH] + a1[:, CH:]
        l = l0[CH:] + l1[:CH] + l1[CH:]
        y[b, CH:] = (acc * inv_ws / l).T
    return y


# revision 61
# speedup vs baseline: 1.0105x; 1.0105x over previous
"""Single-head causal attention (B=4, T=4096, E=1024, H=128) on 8 TRN2 cores.

Sharding: 2 cores per batch, "folded triangle" split of the causal work.
Chunk0 = queries [0,2048), chunk1 = [2048,4096).
  core (b, 0): TRI : chunk0 q vs k in [0, q]        (causal triangle)
               RECT: chunk1 q vs k in [0, 1024)     (no mask)
  core (b, 1): TRI : chunk1 q vs k in [2048, q]     (causal triangle)
               RECT: chunk1 q vs k in [1024, 2048)  (no mask)
All 8 cores run the identical SPMD program; only the data differs. Outputs
are unnormalized accumulators acc = P@V and row-sums l = P@1; the host sums
chunk1 partials and normalizes y = acc/(32*l).

fp8 fast path: x and W ship as fp8e4 with W pre-scaled by 32 (so W' is
U(-1,1), in e4m3's clean range), and projections run as fp8 DoubleRow
matmuls (2 E-chunks per pass, 2x PE rate). V^T is produced directly by the
PE (x-slice as the stationary operand) so no transposes are needed. Q/K
stay bf16 for the S = K^T Q matmul (contraction is 128, so DoubleRow cannot
pair there). The exp runs on ScalarE reading S straight from PSUM at scale
1/(1024*sqrt(128)) and writes P^T as fp8e4; A@V and the row-sum matmuls are
then fp8 DoubleRow over paired k-tiles. The causal mask is applied POST-exp
on the (otherwise idle) GPSIMD engine as an affine_select fill of 0 on pt;
fully-masked diagonal columns are skipped in both the S matmuls and the
exp. Selective precision: the first 512 rows of each core's triangle chunk
(the rows whose softmax has too few terms to average out fp8 noise) use a
bf16 path end-to-end -- bf16 x/W block-0 projections (qtBF/ktBF/vsbBF) and
bf16 P/AV/l for that group -- which recovers reference-level accuracy
(rel err ~3.7e-3 vs 5.2e-2 all-fp8).

Schedule: attention is a software-pipelined stream of 36 pair-jobs
(S -> exp -> mask -> AV+l), group order (qb4, qb1, qb2, qb3, qb5, qb6,
then qb0's two bf16 jobs interleaved inside qb7's four fp8 jobs so the
fp8 stream hides the wait for the bf16 trio's evacuations), with
projection units interleaved one-per-slot. PSUM: psS 2x2
banks (S tiles + bf16-unit scratch), psY 2x1 (AV accumulators + V^T units),
psT 2x1 (q/k proj + row-sum accumulators); unit placement follows a strict
rotation audit so no long-lived tile ever blocks the pool rotation. x
blocks are prefetched up-front on one DMA queue in first-use order.
"""

import sys

if "/opt/trn_rl_repo" not in sys.path:
    sys.path.insert(0, "/opt/trn_rl_repo")

import numpy as np
import ml_dtypes

import concourse.bacc as bacc
import concourse.bass as bass
import concourse.mybir as mybir
from concourse import masks, tile
from concourse.bass_utils import run_bass_kernel_spmd

E = 1024
H = 128
T = 4096
CH = 2048            # chunk length
TQ = 4096            # q tokens per core: [tri own-chunk 2048 | rect chunk1 2048]
RK = 1024            # rect-k region length
NKT_RK = RK // 128   # 8 k-tiles in the rect-k region
WS = 32.0            # host pre-scale on W (and so on q', k', v')
SCALE = 1.0 / np.sqrt(np.float32(H))
SCALE8 = SCALE / (WS * WS)   # exp scale for q'.k' = 1024 * q.k
NEG = -1.0e9                 # masked: NEG*SCALE8 ~ -8.6e3 -> exp == 0

F32 = mybir.dt.float32
BF16 = mybir.dt.bfloat16
F8 = mybir.dt.float8e4
DR = mybir.MatmulPerfMode.DoubleRow

EC = E // 128       # 8 contraction chunks for the projections
NB_Q = TQ // 512    # 8 q blocks
NB_RK = RK // 512   # 2 rect-k blocks

_CACHED = {}
TRACE = False
TRACE_CORES = None
LAST_RESULTS = None


def _build(loop_n=None):
    nc = bacc.Bacc("TRN2", target_bir_lowering=False, debug=False, num_devices=8)
    # host-tiled: [128p, tb, ec, 512] (fp8e4)
    xq_in = nc.dram_tensor("xq_in", [128, NB_Q, EC, 512], F8, kind="ExternalInput").ap()
    xrk_in = nc.dram_tensor("xrk_in", [128, NB_RK, EC, 512], F8, kind="ExternalInput").ap()
    w3_in = nc.dram_tensor("w3_in", [128, 3, EC, H], F8, kind="ExternalInput").ap()
    xb0_bf_in = nc.dram_tensor("xb0_bf_in", [128, EC, 512], BF16, kind="ExternalInput").ap()
    w3_bf_in = nc.dram_tensor("w3_bf_in", [128, 3, EC, H], BF16, kind="ExternalInput").ap()
    acc_out = nc.dram_tensor("acc_out", [H, TQ], F32, kind="ExternalOutput").ap()
    warm_out = nc.dram_tensor("warm_out", [1, 1], F32, kind="ExternalOutput").ap()
    l_out = nc.dram_tensor("l_out", [1, TQ], F32, kind="ExternalOutput").ap()

    import contextlib

    with tile.TileContext(nc) as tc:
        loop_cm = tc.For_i(0, loop_n, 1) if loop_n else contextlib.nullcontext()
        with (
            tc.tile_pool(name="const", bufs=1) as constp,
            tc.tile_pool(name="wpool", bufs=1) as wpool,
            tc.tile_pool(name="xpool", bufs=1) as xpool,
            tc.tile_pool(name="proj", bufs=1) as projp,
            tc.tile_pool(name="ppool", bufs=12) as ppool,
            tc.tile_pool(name="outp", bufs=4) as outp,
            tc.tile_pool(name="psS", bufs=2, space="PSUM") as psS,
            tc.tile_pool(name="psY", bufs=2, space="PSUM") as psY,
            tc.tile_pool(name="psT", bufs=2, space="PSUM") as psT,
            loop_cm,
        ):
            # ---- input DMAs on two queues (SP + DVE) for parallel streams
            xtiles = {}

            def xdma(eng, key, src_, tb):
                xb = xpool.tile([128, EC, 512], F8, tag=f"x{key}", name=f"x{key}")
                eng.dma_start(xb[:], src_[:, tb])
                xtiles[key] = xb

            w3 = wpool.tile([128, 3, EC, H], F8, tag="w3")
            nc.sync.dma_start(w3[:], w3_in)
            wtiles = {name: w3[:, i] for i, name in enumerate(("q", "k", "v"))}
            xdma(nc.sync, "q4", xq_in, 4)
            xdma(nc.sync, "rk0", xrk_in, 0)
            xdma(nc.sync, "rk1", xrk_in, 1)
            xdma(nc.sync, "q0", xq_in, 0)
            xq1 = xpool.tile([128, EC, 512], F8, tag="xq1", name="xq1")
            nc.sync.dma_start(xq1[:, :4, :], xq_in[:, 1, :4, :])
            nc.sync.dma_start(xq1[:, 4:, :], xq_in[:, 1, 4:, :])
            xtiles["q1"] = xq1
            xdma(nc.sync, "q2", xq_in, 2)
            w3bf = wpool.tile([128, 3, EC, H], BF16, tag="w3bf")
            nc.sync.dma_start(w3bf[:], w3_bf_in)
            xb0bf = xpool.tile([128, EC, 512], BF16, tag="xb0bf", name="xb0bf")
            nc.sync.dma_start(xb0bf[:, :4, :], xb0_bf_in[:, :4, :])
            nc.sync.dma_start(xb0bf[:, 4:, :], xb0_bf_in[:, 4:, :])
            wbf = {name: w3bf[:, i] for i, name in enumerate(("q", "k", "v"))}
            xdma(nc.sync, "q3", xq_in, 3)
            xdma(nc.sync, "q5", xq_in, 5)
            xdma(nc.sync, "q6", xq_in, 6)
            xdma(nc.sync, "q7", xq_in, 7)

            # ---- constants (Pool engine; overlaps the DMAs) ----
            ones_f = constp.tile([128, 1], F32, tag="ones32")
            nc.gpsimd.memset(ones_f[:], 1.0)
            ones8 = constp.tile([128, 2, 32], F8, tag="ones8")
            nc.gpsimd.memset(ones8[:], 1.0)
            ones_bf = constp.tile([128, 1], BF16, tag="ones_bf")
            nc.vector.tensor_copy(ones_bf[:], ones_f[:])
            warm = constp.tile([1, 1], F32, tag="warm")
            nc.scalar.activation(
                warm[:], ones_f[:1, :1], mybir.ActivationFunctionType.Exp, scale=1.0
            )
            nc.sync.dma_start(warm_out, warm[:])

            # ---- projection targets ----
            # QT [128h, TQ]; KT [128h, 3072] (bf16); V^T [token, k-tile, h] f8
            # kv tile space: tiles 0..7 = rect-k, 8..23 = tri chunk
            qt = projp.tile([128, TQ], BF16, tag="qt")
            kt = projp.tile([128, RK + CH], BF16, tag="kt")
            ktBF = projp.tile([128, 512], BF16, tag="ktBF")   # bf16 k, tiles 8-11
            vsb = projp.tile([128, NKT_RK + CH // 128, 128], F8, tag="v")
            vsbBF = projp.tile([128, 4, 128], BF16, tag="vbf")

            def qk_unit(name, xb, dcol):
                """f8 DoubleRow projection of q or k; evacuate to qt/kt."""
                ps = psT.tile([128, 512], F32, tag="psproj")
                for e2 in range(EC // 2):
                    nc.tensor.matmul(
                        ps[:], wtiles[name][:, 2 * e2 : 2 * e2 + 2, :],
                        xb[:, 2 * e2 : 2 * e2 + 2, :],
                        start=(e2 == 0), stop=(e2 == EC // 2 - 1),
                        perf_mode=DR,
                    )
                dst = kt if name == "k" else qt
                nc.vector.tensor_copy(dst[:, dcol : dcol + 512], ps[:])

            def v_unit(xb, kt0):
                """f8 V^T computed directly: out[token, h] = x_sliceT @ Wv.
                Allocates from psY: v-units interleave cleanly with ys tiles."""
                ps = psY.tile([128, 512], F32, tag="y", name="psv")
                for tj in range(4):
                    for e2 in range(EC // 2):
                        nc.tensor.matmul(
                            ps[:, 128 * tj : 128 * (tj + 1)],
                            xb[:, 2 * e2 : 2 * e2 + 2, 128 * tj : 128 * (tj + 1)],
                            wtiles["v"][:, 2 * e2 : 2 * e2 + 2, :],
                            start=(e2 == 0), stop=(e2 == EC // 2 - 1),
                            perf_mode=DR,
                        )
                nc.vector.tensor_copy(vsb[:, kt0 : kt0 + 4, :], ps[:])

            # bf16 block-0 units, each emitted as 4 chunks spread over
            # consecutive job slots (the psum alloc happens in chunk 0 at the
            # rotation-audited slot; later chunks add matmuls only).
            _bfst = {}

            def qkbf_chunk(name, dst, c):
                if c == 0:
                    _bfst[name] = psT.tile([128, 512], F32, tag="psproj",
                                           name=f"ps{name}bf")
                ps = _bfst[name]
                for ec in (2 * c, 2 * c + 1):
                    nc.tensor.matmul(ps[:], wbf[name][:, ec, :],
                                     xb0bf[:, ec, :],
                                     start=(ec == 0), stop=(ec == EC - 1))
                if c == 3:
                    nc.vector.tensor_copy(dst, ps[:])

            def qbf_chunk(c):
                qkbf_chunk("q", qt[:, 0:512], c)

            def kbf_chunk(c):
                qkbf_chunk("k", ktBF[:], c)

            def vbf_chunk(c):
                if c == 0:
                    _bfst["v"] = psY.tile([128, 512], F32, tag="y",
                                          name="psvbf")
                ps = _bfst["v"]
                tj = c
                for ec in range(EC):
                    nc.tensor.matmul(
                        ps[:, 128 * tj : 128 * (tj + 1)],
                        xb0bf[:, ec, 128 * tj : 128 * (tj + 1)],
                        wbf["v"][:, ec, :],
                        start=(ec == 0), stop=(ec == EC - 1))
                if c == 3:
                    nc.vector.tensor_copy(vsbBF[:], ps[:])

            # ---- attention jobs; group order keeps the bf16 block off the
            # DMA critical path and respects the psT rotation audit ----
            # job = (qb, k0, d0, k1, d1, first_in_block, last_in_block)
            gjobs = {}
            for qb in range(8):
                gjobs[qb] = []
                if qb < 4:  # triangle over kv tiles 8..(8+4qb+4)
                    kts = []
                    for j in range(4 * qb + 4):
                        dp = j - 4 * qb if j >= 4 * qb else None
                        kts.append((NKT_RK + j, dp))
                    for i in range(len(kts) // 2):
                        (k0, d0), (k1, d1) = kts[2 * i], kts[2 * i + 1]
                        gjobs[qb].append((qb, k0, d0, k1, d1, i == 0,
                                          2 * i + 2 == len(kts)))
                else:  # rect over kv tiles 0..7
                    for i in range(NKT_RK // 2):
                        gjobs[qb].append((qb, 2 * i, None, 2 * i + 1, None,
                                          i == 0, 2 * i + 2 == NKT_RK))
            # tail: interleave qb0's two bf16 jobs inside qb7 so the fp8
            # stream hides the wait for the bf16 trio's evacuations
            jobs = (gjobs[4] + gjobs[1] + gjobs[2] + gjobs[3] + gjobs[5]
                    + gjobs[6] + [gjobs[7][0], gjobs[0][0], gjobs[7][1],
                                  gjobs[0][1], gjobs[7][2], gjobs[7][3]])

            n = len(jobs)
            ss_t = [None] * n
            pt_t = [None] * n
            ybank = {}

            def s_stage(j):
                qb, k0, d0, k1, d1, _, _ = jobs[j]
                ss = psS.tile([128, 2, 512], F32, tag="s")
                ss_t[j] = ss
                qs = qt[:, 512 * qb : 512 * (qb + 1)]
                if qb == 0:
                    kt0 = ktBF[:, 128 * (k0 - NKT_RK) : 128 * (k0 - NKT_RK + 1)]
                    kt1 = ktBF[:, 128 * (k1 - NKT_RK) : 128 * (k1 - NKT_RK + 1)]
                else:
                    kt0 = kt[:, 128 * k0 : 128 * (k0 + 1)]
                    kt1 = kt[:, 128 * k1 : 128 * (k1 + 1)]
                # fully-masked diag columns [0, 128*d) are never computed:
                # the Pool mask fill overwrites them in pt afterwards.
                c0 = 128 * d0 if d0 else 0
                c1 = 128 * d1 if d1 else 0
                nc.tensor.matmul(ss[:, 0, c0:], kt0, qs[:, c0:],
                                 start=True, stop=True)
                nc.tensor.matmul(ss[:, 1, c1:], kt1, qs[:, c1:],
                                 start=True, stop=True)
                pt = ppool.tile([128, 2, 512], BF16 if qb == 0 else F8, tag="pt")
                pt_t[j] = pt
                if d1 is not None and d1 >= 2:
                    # (2,3) diag pair: one narrowed exp over both halves'
                    # live region [256, 512) + [384->256...]: use col range
                    # [c0, 512) of both halves in a single activation
                    nc.scalar.activation(
                        pt[:, :, c0:], ss[:, :, c0:],
                        mybir.ActivationFunctionType.Exp, scale=SCALE8)
                else:
                    nc.scalar.activation(
                        pt[:], ss[:], mybir.ActivationFunctionType.Exp,
                        scale=SCALE8)
                # causal mask applied post-exp on the (idle) Pool engine:
                # zero pt where q < k + 128*d; only cols [0, 128*(d+1)) affected
                if d0 is not None:
                    w = 128 * (d0 + 1)
                    nc.gpsimd.affine_select(
                        out=pt[:, 0, :w], in_=pt[:, 0, :w],
                        compare_op=mybir.AluOpType.is_ge,
                        fill=0.0, base=-128 * d0,
                        pattern=[[1, w]], channel_multiplier=-1,
                    )
                if d1 is not None:
                    w = 128 * (d1 + 1)
                    nc.gpsimd.affine_select(
                        out=pt[:, 1, :w], in_=pt[:, 1, :w],
                        compare_op=mybir.AluOpType.is_ge,
                        fill=0.0, base=-128 * d1,
                        pattern=[[1, w]], channel_multiplier=-1,
                    )

            def av_stage(j):
                qb, k0, d0, k1, d1, first, last = jobs[j]
                if first:
                    ys = psY.tile([128, 512], F32, tag="y", name=f"ys{qb}")
                    ls = psT.tile([32, 512], F32, tag="psproj", name=f"ls{qb}")
                    ybank[qb] = (ys, ls)
                ys, ls = ybank[qb]
                pt = pt_t[j]
                if qb == 0:
                    nc.tensor.matmul(ys[:], vsbBF[:, k0 - NKT_RK, :], pt[:, 0, :],
                                     start=first, stop=False)
                    nc.tensor.matmul(ys[:], vsbBF[:, k1 - NKT_RK, :], pt[:, 1, :],
                                     start=False, stop=last)
                    nc.tensor.matmul(ls[:1, :], ones_bf[:], pt[:, 0, :],
                                     start=first, stop=False)
                    nc.tensor.matmul(ls[:1, :], ones_bf[:], pt[:, 1, :],
                                     start=False, stop=last)
                else:
                    nc.tensor.matmul(ys[:], vsb[:, k0 : k0 + 2, :], pt[:],
                                     start=first, stop=last, perf_mode=DR)
                    nc.tensor.matmul(ls[:], ones8[:], pt[:],
                                     start=first, stop=last, perf_mode=DR)
                if last:
                    yo = outp.tile([128, 512], F32, tag="yo")
                    nc.vector.tensor_copy(yo[:], ys[:])
                    nc.sync.dma_start(acc_out[:, 512 * qb : 512 * (qb + 1)], yo[:])
                    lo = outp.tile([1, 512], F32, tag="lo")
                    nc.vector.tensor_copy(lo[:], ls[:1, :])
                    nc.sync.dma_start(l_out[:, 512 * qb : 512 * (qb + 1)], lo[:])

            # unit schedule (see psT rotation audit): kv chunks at group-last
            # slots, q units at +1 after each ls allocation.
            before_s = {
                4: [lambda: qk_unit("q", xtiles["q1"], 512)],
                8: [lambda: qk_unit("q", xtiles["q2"], 1024)],
                14: [lambda: qk_unit("q", xtiles["q3"], 1536)],
                22: [lambda: qk_unit("q", xtiles["q5"], 2560)],
                26: [lambda: qk_unit("q", xtiles["q6"], 3072)],
                30: [lambda: qk_unit("q", xtiles["q7"], 3584)],
            }
            after_av = {
                2: [lambda: v_unit(xtiles["q0"], NKT_RK)],
                3: [lambda: qk_unit("k", xtiles["q1"], RK + 512),
                    lambda: v_unit(xtiles["q1"], NKT_RK + 4)],
                7: [lambda: qk_unit("k", xtiles["q2"], RK + 1024),
                    lambda: v_unit(xtiles["q2"], NKT_RK + 8)],
                13: [lambda: qk_unit("k", xtiles["q3"], RK + 1536),
                     lambda: v_unit(xtiles["q3"], NKT_RK + 12)],
                21: [lambda: qbf_chunk(0)],
                22: [lambda: qbf_chunk(1)],
                23: [lambda: qbf_chunk(2)],
                24: [lambda: qbf_chunk(3)],
                25: [lambda: kbf_chunk(0)],
                26: [lambda: kbf_chunk(1), lambda: vbf_chunk(0)],
                27: [lambda: kbf_chunk(2), lambda: vbf_chunk(1)],
                28: [lambda: kbf_chunk(3), lambda: vbf_chunk(2)],
                29: [lambda: vbf_chunk(3)],
            }

            # prologue: q(b4) + rect-k region (all f8), first S pair, then
            # the b0 f8 k unit (its V rides at after_av[3] in the psY slot).
            qk_unit("q", xtiles["q4"], 2048)
            qk_unit("k", xtiles["rk0"], 0)
            s_stage(0)
            s_stage(1)
            v_unit(xtiles["rk0"], 0)
            qk_unit("k", xtiles["rk1"], 512)
            v_unit(xtiles["rk1"], 4)
            qk_unit("k", xtiles["q0"], RK)
            av_stage(0)
            for j in range(1, n):
                if j + 1 < n:
                    for u in before_s.get(j + 1, []):
                        u()
                    s_stage(j + 1)
                av_stage(j)
                for u in after_av.get(j, []):
                    u()

    nc.compile()
    return nc


def _prep_x(xpart):
    """[Tpart, E] f32 -> fp8e4 tiled [128, tb, ec, 512] host layout."""
    tb = xpart.shape[0] // 512
    a = xpart.T.astype(ml_dtypes.float8_e4m3)       # [E, Tpart]
    a = a.reshape(EC, 128, tb, 512).transpose(1, 2, 0, 3)
    return np.ascontiguousarray(a)


def _prep_w(w, dt=None):
    """[H, E] f32 -> [128, ec, H] (32 * w.T chunked) in dt (default fp8e4)."""
    a = (w.T * WS).astype(dt or ml_dtypes.float8_e4m3)  # [E, H]
    a = a.reshape(EC, 128, H).transpose(1, 0, 2)
    return np.ascontiguousarray(a)


def _prep_xbf(xpart):
    """[512, E] f32 -> bf16 tiled [128, ec, 512]."""
    a = xpart.T.astype(ml_dtypes.bfloat16)          # [E, 512]
    a = a.reshape(EC, 128, 512).transpose(1, 0, 2)
    return np.ascontiguousarray(a)


def kernel(x_in, Wq, Wk, Wv):
    B, T_, E_ = x_in.shape
    assert (B, T_, E_) == (4, T, E)
    nc = _CACHED.get("nc")
    if nc is None:
        nc = _CACHED["nc"] = _build()

    bf = ml_dtypes.bfloat16
    w3 = np.ascontiguousarray(np.stack([_prep_w(W) for W in (Wq, Wk, Wv)], axis=1))
    w3b = np.ascontiguousarray(
        np.stack([_prep_w(W, bf) for W in (Wq, Wk, Wv)], axis=1))
    in_maps = []
    for c in range(8):
        b, h = c // 2, c % 2
        xb = np.asarray(x_in[b], dtype=np.float32)
        c0, c1 = xb[:CH], xb[CH:]
        own = c0 if h == 0 else c1
        xq = np.concatenate([own, c1], axis=0)        # [4096, E]
        rk = xb[0:RK] if h == 0 else xb[RK : 2 * RK]  # [1024, E]
        in_maps.append(
            {"xq_in": _prep_x(xq), "xrk_in": _prep_x(rk),
             "xb0_bf_in": _prep_xbf(xq[:512]),
             "w3_in": w3, "w3_bf_in": w3b}
        )

    kw = {}
    if TRACE:
        kw = {"trace": True, "trace_cores": TRACE_CORES}
    res = run_bass_kernel_spmd(nc, in_maps, core_ids=list(range(8)), **kw)
    global LAST_RESULTS
    LAST_RESULTS = res

    y = np.empty((B, T, H), dtype=np.float32)
    inv_ws = 1.0 / WS
    for b in range(4):
        r0, r1 = res.results[2 * b], res.results[2 * b + 1]
        a0, l0 = r0["acc_out"], r0["l_out"][0]
        a1, l1 = r1["acc_out"], r1["l_out"][0]
        y[b, :CH] = (a0[:, :CH] * inv_ws / l0[:CH]).T
        acc = a0[:, CH:] + a1[:, :CH] + a1[:, CH:]
        l = l0[CH:] + l1[:CH] + l1[CH:]
        y[b, CH:] = (acc * inv_ws / l).T
    return y


# revision 62
# speedup vs baseline: 1.0116x; 1.0010x over previous
"""Single-head causal attention (B=4, T=4096, E=1024, H=128) on 8 TRN2 cores.

Sharding: 2 cores per batch, "folded triangle" split of the causal work.
Chunk0 = queries [0,2048), chunk1 = [2048,4096).
  core (b, 0): TRI : chunk0 q vs k in [0, q]        (causal triangle)
               RECT: chunk1 q vs k in [0, 1024)     (no mask)
  core (b, 1): TRI : chunk1 q vs k in [2048, q]     (causal triangle)
               RECT: chunk1 q vs k in [1024, 2048)  (no mask)
All 8 cores run the identical SPMD program; only the data differs. Outputs
are unnormalized accumulators acc = P@V and row-sums l = P@1; the host sums
chunk1 partials and normalizes y = acc/(32*l).

fp8 fast path: x and W ship as fp8e4 with W pre-scaled by 32 (so W' is
U(-1,1), in e4m3's clean range), and projections run as fp8 DoubleRow
matmuls (2 E-chunks per pass, 2x PE rate). V^T is produced directly by the
PE (x-slice as the stationary operand) so no transposes are needed. Q/K
stay bf16 for the S = K^T Q matmul (contraction is 128, so DoubleRow cannot
pair there). The exp runs on ScalarE reading S straight from PSUM at scale
1/(1024*sqrt(128)) and writes P^T as fp8e4; A@V and the row-sum matmuls are
then fp8 DoubleRow over paired k-tiles. The causal mask is applied POST-exp
on the (otherwise idle) GPSIMD engine as an affine_select fill of 0 on pt;
fully-masked diagonal columns are skipped in both the S matmuls and the
exp. Selective precision: the first 512 rows of each core's triangle chunk
(the rows whose softmax has too few terms to average out fp8 noise) use a
bf16 path end-to-end -- bf16 x/W block-0 projections (qtBF/ktBF/vsbBF) and
bf16 P/AV/l for that group -- which recovers reference-level accuracy
(rel err ~3.7e-3 vs 5.2e-2 all-fp8).

Schedule: attention is a software-pipelined stream of 36 pair-jobs
(S -> exp -> mask -> AV+l), group order (qb4, qb1, qb2, qb3, qb5, qb6,
then qb0's two bf16 jobs interleaved inside qb7's four fp8 jobs so the
fp8 stream hides the wait for the bf16 trio's evacuations), with
projection units interleaved one-per-slot. PSUM: psS 2x2
banks (S tiles + bf16-unit scratch), psY 2x1 (AV accumulators + V^T units),
psT 2x1 (q/k proj + row-sum accumulators); unit placement follows a strict
rotation audit so no long-lived tile ever blocks the pool rotation. x
blocks are prefetched up-front on one DMA queue in first-use order.
"""

import sys

if "/opt/trn_rl_repo" not in sys.path:
    sys.path.insert(0, "/opt/trn_rl_repo")

import numpy as np
import ml_dtypes

import concourse.bacc as bacc
import concourse.bass as bass
import concourse.mybir as mybir
from concourse import masks, tile
from concourse.bass_utils import run_bass_kernel_spmd

E = 1024
H = 128
T = 4096
CH = 2048            # chunk length
TQ = 4096            # q tokens per core: [tri own-chunk 2048 | rect chunk1 2048]
RK = 1024            # rect-k region length
NKT_RK = RK // 128   # 8 k-tiles in the rect-k region
WS = 32.0            # host pre-scale on W (and so on q', k', v')
SCALE = 1.0 / np.sqrt(np.float32(H))
SCALE8 = SCALE / (WS * WS)   # exp scale for q'.k' = 1024 * q.k
NEG = -1.0e9                 # masked: NEG*SCALE8 ~ -8.6e3 -> exp == 0

F32 = mybir.dt.float32
BF16 = mybir.dt.bfloat16
F8 = mybir.dt.float8e4
DR = mybir.MatmulPerfMode.DoubleRow

EC = E // 128       # 8 contraction chunks for the projections
NB_Q = TQ // 512    # 8 q blocks
NB_RK = RK // 512   # 2 rect-k blocks

_CACHED = {}
TRACE = False
TRACE_CORES = None
LAST_RESULTS = None


def _build(loop_n=None):
    nc = bacc.Bacc("TRN2", target_bir_lowering=False, debug=False, num_devices=8)
    # host-tiled: [128p, tb, ec, 512] (fp8e4)
    xq_in = nc.dram_tensor("xq_in", [128, NB_Q, EC, 512], F8, kind="ExternalInput").ap()
    xrk_in = nc.dram_tensor("xrk_in", [128, NB_RK, EC, 512], F8, kind="ExternalInput").ap()
    w3_in = nc.dram_tensor("w3_in", [128, 3, EC, H], F8, kind="ExternalInput").ap()
    xb0_bf_in = nc.dram_tensor("xb0_bf_in", [128, EC, 512], BF16, kind="ExternalInput").ap()
    w3_bf_in = nc.dram_tensor("w3_bf_in", [128, 3, EC, H], BF16, kind="ExternalInput").ap()
    acc_out = nc.dram_tensor("acc_out", [H, TQ], F32, kind="ExternalOutput").ap()
    warm_out = nc.dram_tensor("warm_out", [1, 1], F32, kind="ExternalOutput").ap()
    l_out = nc.dram_tensor("l_out", [1, TQ], F32, kind="ExternalOutput").ap()

    import contextlib

    with tile.TileContext(nc) as tc:
        loop_cm = tc.For_i(0, loop_n, 1) if loop_n else contextlib.nullcontext()
        with (
            tc.tile_pool(name="const", bufs=1) as constp,
            tc.tile_pool(name="wpool", bufs=1) as wpool,
            tc.tile_pool(name="xpool", bufs=1) as xpool,
            tc.tile_pool(name="proj", bufs=1) as projp,
            tc.tile_pool(name="ppool", bufs=16) as ppool,
            tc.tile_pool(name="outp", bufs=6) as outp,
            tc.tile_pool(name="psS", bufs=2, space="PSUM") as psS,
            tc.tile_pool(name="psY", bufs=2, space="PSUM") as psY,
            tc.tile_pool(name="psT", bufs=2, space="PSUM") as psT,
            loop_cm,
        ):
            # ---- input DMAs on two queues (SP + DVE) for parallel streams
            xtiles = {}

            def xdma(eng, key, src_, tb):
                xb = xpool.tile([128, EC, 512], F8, tag=f"x{key}", name=f"x{key}")
                eng.dma_start(xb[:], src_[:, tb])
                xtiles[key] = xb

            w3 = wpool.tile([128, 3, EC, H], F8, tag="w3")
            nc.sync.dma_start(w3[:], w3_in)
            wtiles = {name: w3[:, i] for i, name in enumerate(("q", "k", "v"))}
            xdma(nc.sync, "q4", xq_in, 4)
            xdma(nc.sync, "rk0", xrk_in, 0)
            xdma(nc.sync, "rk1", xrk_in, 1)
            xdma(nc.sync, "q0", xq_in, 0)
            xq1 = xpool.tile([128, EC, 512], F8, tag="xq1", name="xq1")
            nc.sync.dma_start(xq1[:, :4, :], xq_in[:, 1, :4, :])
            nc.sync.dma_start(xq1[:, 4:, :], xq_in[:, 1, 4:, :])
            xtiles["q1"] = xq1
            xdma(nc.sync, "q2", xq_in, 2)
            w3bf = wpool.tile([128, 3, EC, H], BF16, tag="w3bf")
            nc.sync.dma_start(w3bf[:], w3_bf_in)
            xb0bf = xpool.tile([128, EC, 512], BF16, tag="xb0bf", name="xb0bf")
            nc.sync.dma_start(xb0bf[:, :4, :], xb0_bf_in[:, :4, :])
            nc.sync.dma_start(xb0bf[:, 4:, :], xb0_bf_in[:, 4:, :])
            wbf = {name: w3bf[:, i] for i, name in enumerate(("q", "k", "v"))}
            xdma(nc.sync, "q3", xq_in, 3)
            xdma(nc.sync, "q5", xq_in, 5)
            xdma(nc.sync, "q6", xq_in, 6)
            xdma(nc.sync, "q7", xq_in, 7)

            # ---- constants (Pool engine; overlaps the DMAs) ----
            ones_f = constp.tile([128, 1], F32, tag="ones32")
            nc.gpsimd.memset(ones_f[:], 1.0)
            ones8 = constp.tile([128, 2, 32], F8, tag="ones8")
            nc.gpsimd.memset(ones8[:], 1.0)
            ones_bf = constp.tile([128, 1], BF16, tag="ones_bf")
            nc.vector.tensor_copy(ones_bf[:], ones_f[:])
            warm = constp.tile([1, 1], F32, tag="warm")
            nc.scalar.activation(
                warm[:], ones_f[:1, :1], mybir.ActivationFunctionType.Exp, scale=1.0
            )
            nc.sync.dma_start(warm_out, warm[:])

            # ---- projection targets ----
            # QT [128h, TQ]; KT [128h, 3072] (bf16); V^T [token, k-tile, h] f8
            # kv tile space: tiles 0..7 = rect-k, 8..23 = tri chunk
            qt = projp.tile([128, TQ], BF16, tag="qt")
            kt = projp.tile([128, RK + CH], BF16, tag="kt")
            ktBF = projp.tile([128, 512], BF16, tag="ktBF")   # bf16 k, tiles 8-11
            vsb = projp.tile([128, NKT_RK + CH // 128, 128], F8, tag="v")
            vsbBF = projp.tile([128, 4, 128], BF16, tag="vbf")

            def qk_unit(name, xb, dcol):
                """f8 DoubleRow projection of q or k; evacuate to qt/kt."""
                ps = psT.tile([128, 512], F32, tag="psproj")
                for e2 in range(EC // 2):
                    nc.tensor.matmul(
                        ps[:], wtiles[name][:, 2 * e2 : 2 * e2 + 2, :],
                        xb[:, 2 * e2 : 2 * e2 + 2, :],
                        start=(e2 == 0), stop=(e2 == EC // 2 - 1),
                        perf_mode=DR,
                    )
                dst = kt if name == "k" else qt
                nc.vector.tensor_copy(dst[:, dcol : dcol + 512], ps[:])

            def v_unit(xb, kt0):
                """f8 V^T computed directly: out[token, h] = x_sliceT @ Wv.
                Allocates from psY: v-units interleave cleanly with ys tiles."""
                ps = psY.tile([128, 512], F32, tag="y", name="psv")
                for tj in range(4):
                    for e2 in range(EC // 2):
                        nc.tensor.matmul(
                            ps[:, 128 * tj : 128 * (tj + 1)],
                            xb[:, 2 * e2 : 2 * e2 + 2, 128 * tj : 128 * (tj + 1)],
                            wtiles["v"][:, 2 * e2 : 2 * e2 + 2, :],
                            start=(e2 == 0), stop=(e2 == EC // 2 - 1),
                            perf_mode=DR,
                        )
                nc.vector.tensor_copy(vsb[:, kt0 : kt0 + 4, :], ps[:])

            # bf16 block-0 units, each emitted as 4 chunks spread over
            # consecutive job slots (the psum alloc happens in chunk 0 at the
            # rotation-audited slot; later chunks add matmuls only).
            _bfst = {}

            def qkbf_chunk(name, dst, c):
                if c == 0:
                    _bfst[name] = psT.tile([128, 512], F32, tag="psproj",
                                           name=f"ps{name}bf")
                ps = _bfst[name]
                for ec in (2 * c, 2 * c + 1):
                    nc.tensor.matmul(ps[:], wbf[name][:, ec, :],
                                     xb0bf[:, ec, :],
                                     start=(ec == 0), stop=(ec == EC - 1))
                if c == 3:
                    nc.vector.tensor_copy(dst, ps[:])

            def qbf_chunk(c):
                qkbf_chunk("q", qt[:, 0:512], c)

            def kbf_chunk(c):
                qkbf_chunk("k", ktBF[:], c)

            def vbf_chunk(c):
                if c == 0:
                    _bfst["v"] = psY.tile([128, 512], F32, tag="y",
                                          name="psvbf")
                ps = _bfst["v"]
                tj = c
                for ec in range(EC):
                    nc.tensor.matmul(
                        ps[:, 128 * tj : 128 * (tj + 1)],
                        xb0bf[:, ec, 128 * tj : 128 * (tj + 1)],
                        wbf["v"][:, ec, :],
                        start=(ec == 0), stop=(ec == EC - 1))
                if c == 3:
                    nc.vector.tensor_copy(vsbBF[:], ps[:])

            # ---- attention jobs; group order keeps the bf16 block off the
            # DMA critical path and respects the psT rotation audit ----
            # job = (qb, k0, d0, k1, d1, first_in_block, last_in_block)
            gjobs = {}
            for qb in range(8):
                gjobs[qb] = []
                if qb < 4:  # triangle over kv tiles 8..(8+4qb+4)
                    kts = []
                    for j in range(4 * qb + 4):
                        dp = j - 4 * qb if j >= 4 * qb else None
                        kts.append((NKT_RK + j, dp))
                    for i in range(len(kts) // 2):
                        (k0, d0), (k1, d1) = kts[2 * i], kts[2 * i + 1]
                        gjobs[qb].append((qb, k0, d0, k1, d1, i == 0,
                                          2 * i + 2 == len(kts)))
                else:  # rect over kv tiles 0..7
                    for i in range(NKT_RK // 2):
                        gjobs[qb].append((qb, 2 * i, None, 2 * i + 1, None,
                                          i == 0, 2 * i + 2 == NKT_RK))
            # tail: interleave qb0's two bf16 jobs inside qb7 so the fp8
            # stream hides the wait for the bf16 trio's evacuations
            jobs = (gjobs[4] + gjobs[1] + gjobs[2] + gjobs[3] + gjobs[5]
                    + gjobs[6] + [gjobs[7][0], gjobs[0][0], gjobs[7][1],
                                  gjobs[0][1], gjobs[7][2], gjobs[7][3]])

            n = len(jobs)
            ss_t = [None] * n
            pt_t = [None] * n
            ybank = {}

            def s_stage(j):
                qb, k0, d0, k1, d1, _, _ = jobs[j]
                ss = psS.tile([128, 2, 512], F32, tag="s")
                ss_t[j] = ss
                qs = qt[:, 512 * qb : 512 * (qb + 1)]
                if qb == 0:
                    kt0 = ktBF[:, 128 * (k0 - NKT_RK) : 128 * (k0 - NKT_RK + 1)]
                    kt1 = ktBF[:, 128 * (k1 - NKT_RK) : 128 * (k1 - NKT_RK + 1)]
                else:
                    kt0 = kt[:, 128 * k0 : 128 * (k0 + 1)]
                    kt1 = kt[:, 128 * k1 : 128 * (k1 + 1)]
                # fully-masked diag columns [0, 128*d) are never computed:
                # the Pool mask fill overwrites them in pt afterwards.
                c0 = 128 * d0 if d0 else 0
                c1 = 128 * d1 if d1 else 0
                nc.tensor.matmul(ss[:, 0, c0:], kt0, qs[:, c0:],
                                 start=True, stop=True)
                nc.tensor.matmul(ss[:, 1, c1:], kt1, qs[:, c1:],
                                 start=True, stop=True)
                pt = ppool.tile([128, 2, 512], BF16 if qb == 0 else F8, tag="pt")
                pt_t[j] = pt
                if d1 is not None and d1 >= 2:
                    # (2,3) diag pair: one narrowed exp over both halves'
                    # live region [256, 512) + [384->256...]: use col range
                    # [c0, 512) of both halves in a single activation
                    nc.scalar.activation(
                        pt[:, :, c0:], ss[:, :, c0:],
                        mybir.ActivationFunctionType.Exp, scale=SCALE8)
                else:
                    nc.scalar.activation(
                        pt[:], ss[:], mybir.ActivationFunctionType.Exp,
                        scale=SCALE8)
                # causal mask applied post-exp on the (idle) Pool engine:
                # zero pt where q < k + 128*d; only cols [0, 128*(d+1)) affected
                if d0 is not None:
                    w = 128 * (d0 + 1)
                    nc.gpsimd.affine_select(
                        out=pt[:, 0, :w], in_=pt[:, 0, :w],
                        compare_op=mybir.AluOpType.is_ge,
                        fill=0.0, base=-128 * d0,
                        pattern=[[1, w]], channel_multiplier=-1,
                    )
                if d1 is not None:
                    w = 128 * (d1 + 1)
                    nc.gpsimd.affine_select(
                        out=pt[:, 1, :w], in_=pt[:, 1, :w],
                        compare_op=mybir.AluOpType.is_ge,
                        fill=0.0, base=-128 * d1,
                        pattern=[[1, w]], channel_multiplier=-1,
                    )

            def av_stage(j):
                qb, k0, d0, k1, d1, first, last = jobs[j]
                if first:
                    ys = psY.tile([128, 512], F32, tag="y", name=f"ys{qb}")
                    ls = psT.tile([32, 512], F32, tag="psproj", name=f"ls{qb}")
                    ybank[qb] = (ys, ls)
                ys, ls = ybank[qb]
                pt = pt_t[j]
                if qb == 0:
                    nc.tensor.matmul(ys[:], vsbBF[:, k0 - NKT_RK, :], pt[:, 0, :],
                                     start=first, stop=False)
                    nc.tensor.matmul(ys[:], vsbBF[:, k1 - NKT_RK, :], pt[:, 1, :],
                                     start=False, stop=last)
                    nc.tensor.matmul(ls[:1, :], ones_bf[:], pt[:, 0, :],
                                     start=first, stop=False)
                    nc.tensor.matmul(ls[:1, :], ones_bf[:], pt[:, 1, :],
                                     start=False, stop=last)
                else:
                    nc.tensor.matmul(ys[:], vsb[:, k0 : k0 + 2, :], pt[:],
                                     start=first, stop=last, perf_mode=DR)
                    nc.tensor.matmul(ls[:], ones8[:], pt[:],
                                     start=first, stop=last, perf_mode=DR)
                if last:
                    yo = outp.tile([128, 512], F32, tag="yo")
                    nc.vector.tensor_copy(yo[:], ys[:])
                    nc.sync.dma_start(acc_out[:, 512 * qb : 512 * (qb + 1)], yo[:])
                    lo = outp.tile([1, 512], F32, tag="lo")
                    nc.vector.tensor_copy(lo[:], ls[:1, :])
                    nc.sync.dma_start(l_out[:, 512 * qb : 512 * (qb + 1)], lo[:])

            # unit schedule (see psT rotation audit): kv chunks at group-last
            # slots, q units at +1 after each ls allocation.
            before_s = {
                4: [lambda: qk_unit("q", xtiles["q1"], 512)],
                8: [lambda: qk_unit("q", xtiles["q2"], 1024)],
                14: [lambda: qk_unit("q", xtiles["q3"], 1536)],
                22: [lambda: qk_unit("q", xtiles["q5"], 2560)],
                26: [lambda: qk_unit("q", xtiles["q6"], 3072)],
                30: [lambda: qk_unit("q", xtiles["q7"], 3584)],
            }
            after_av = {
                2: [lambda: v_unit(xtiles["q0"], NKT_RK)],
                3: [lambda: qk_unit("k", xtiles["q1"], RK + 512),
                    lambda: v_unit(xtiles["q1"], NKT_RK + 4)],
                7: [lambda: qk_unit("k", xtiles["q2"], RK + 1024),
                    lambda: v_unit(xtiles["q2"], NKT_RK + 8)],
                13: [lambda: qk_unit("k", xtiles["q3"], RK + 1536),
                     lambda: v_unit(xtiles["q3"], NKT_RK + 12)],
                21: [lambda: qbf_chunk(0)],
                22: [lambda: qbf_chunk(1)],
                23: [lambda: qbf_chunk(2)],
                24: [lambda: qbf_chunk(3)],
                25: [lambda: kbf_chunk(0)],
                26: [lambda: kbf_chunk(1), lambda: vbf_chunk(0)],
                27: [lambda: kbf_chunk(2), lambda: vbf_chunk(1)],
                28: [lambda: kbf_chunk(3), lambda: vbf_chunk(2)],
                29: [lambda: vbf_chunk(3)],
            }

            # prologue: q(b4) + rect-k region (all f8), first S pair, then
            # the b0 f8 k unit (its V rides at after_av[3] in the psY slot).
            qk_unit("q", xtiles["q4"], 2048)
            qk_unit("k", xtiles["rk0"], 0)
            s_stage(0)
            s_stage(1)
            v_unit(xtiles["rk0"], 0)
            qk_unit("k", xtiles["rk1"], 512)
            v_unit(xtiles["rk1"], 4)
            qk_unit("k", xtiles["q0"], RK)
            av_stage(0)
            for j in range(1, n):
                if j + 1 < n:
                    for u in before_s.get(j + 1, []):
                        u()
                    s_stage(j + 1)
                av_stage(j)
                for u in after_av.get(j, []):
                    u()

    nc.compile()
    return nc


def _prep_x(xpart):
    """[Tpart, E] f32 -> fp8e4 tiled [128, tb, ec, 512] host layout."""
    tb = xpart.shape[0] // 512
    a = xpart.T.astype(ml_dtypes.float8_e4m3)       # [E, Tpart]
    a = a.reshape(EC, 128, tb, 512).transpose(1, 2, 0, 3)
    return np.ascontiguousarray(a)


def _prep_w(w, dt=None):
    """[H, E] f32 -> [128, ec, H] (32 * w.T chunked) in dt (default fp8e4)."""
    a = (w.T * WS).astype(dt or ml_dtypes.float8_e4m3)  # [E, H]
    a = a.reshape(EC, 128, H).transpose(1, 0, 2)
    return np.ascontiguousarray(a)


def _prep_xbf(xpart):
    """[512, E] f32 -> bf16 tiled [128, ec, 512]."""
    a = xpart.T.astype(ml_dtypes.bfloat16)          # [E, 512]
    a = a.reshape(EC, 128, 512).transpose(1, 0, 2)
    return np.ascontiguousarray(a)


def kernel(x_in, Wq, Wk, Wv):
    B, T_, E_ = x_in.shape
    assert (B, T_, E_) == (4, T, E)
    nc = _CACHED.get("nc")
    if nc is None:
        nc = _CACHED["nc"] = _build()

    bf = ml_dtypes.bfloat16
    w3 = np.ascontiguousarray(np.stack([_prep_w(W) for W in (Wq, Wk, Wv)], axis=1))
    w3b = np.ascontiguousarray(
        np.stack([_prep_w(W, bf) for W in (Wq, Wk, Wv)], axis=1))
    in_maps = []
    for c in range(8):
        b, h = c // 2, c % 2
        xb = np.asarray(x_in[b], dtype=np.float32)
        c0, c1 = xb[:CH], xb[CH:]
        own = c0 if h == 0 else c1
        xq = np.concatenate([own, c1], axis=0)        # [4096, E]
        rk = xb[0:RK] if h == 0 else xb[RK : 2 * RK]  # [1024, E]
        in_maps.append(
            {"xq_in": _prep_x(xq), "xrk_in": _prep_x(rk),
             "xb0_bf_in": _prep_xbf(xq[:512]),
             "w3_in": w3, "w3_bf_in": w3b}
        )

    kw = {}
    if TRACE:
        kw = {"trace": True, "trace_cores": TRACE_CORES}
    res = run_bass_kernel_spmd(nc, in_maps, core_ids=list(range(8)), **kw)
    global LAST_RESULTS
    LAST_RESULTS = res

    y = np.empty((B, T, H), dtype=np.float32)
    inv_ws = 1.0 / WS
    for b in range(4):
        r0, r1 = res.results[2 * b], res.results[2 * b + 1]
        a0, l0 = r0["acc_out"], r0["l_out"][0]
        a1, l1 = r1["acc_out"], r1["l_out"][0]
        y[b, :CH] = (a0[:, :CH] * inv_ws / l0[:CH]).T
        acc = a0[:, CH:] + a1[:, :CH] + a1[:, CH:]
        l = l0[CH:] + l1[:CH] + l1[CH:]
        y[b, CH:] = (acc * inv_ws / l).T
    return y
